# revision 7
# baseline (speedup 1.0000x reference)
"""Trainium2 Bass kernel for nn_BreedingPolicyNet (sparse_attention family).

Reference semantics (per wave, 8 waves):
    present_p1 = x > 0;  present_p2 = present_p1 with target_idx forced False
    allowed[a,b] = p1[a] & p2[b]
    Qi = softmax(where(allowed, logits, -FLT_MAX), axis=1), zeroed where row empty
    offspring[k] = sum_{a,b} x[a] * Qi[a,b] * T[a,b,k]
    x = max(x + offspring, 0)

Key algebraic property exploited: when every x0[i] > 0 and T >= 0, x stays
strictly positive through all waves (offspring >= 0), so the mask — and
therefore Qi — is IDENTICAL in every wave.  Then
    S[a,k] = sum_b Qi[a,b] * T[a,b,k]        (one single pass over T)
    offspring = x @ S                         (tiny per-wave matvec)
This turns 8 full 512MB passes over T into one (the memory roofline).

Distribution: shard T along axis a (contiguous 64MB per core).  Each core
computes its 64 rows of S with PE matmuls, an AllGather assembles the full
[512,512] S on every core, and all cores redundantly run the 8-wave
recurrence on-device.  Output is read from core 0.
"""

import numpy as np

N = 512
NC = 8           # NeuronCores
SH = N // NC     # a-rows per core
NWAVES = 8
AB = 4           # a-rows fetched per DMA (4MB chunks)
NEG_LARGE = float(np.finfo(np.float32).min)

_prog_cache = {}
last_results = None  # stash of BassKernelResults for test harness introspection


def _qi_matrix(logits: np.ndarray, tgt: int) -> np.ndarray:
    """Wave-invariant Qi: row softmax of logits with column `tgt` masked."""
    masked = np.array(logits, dtype=np.float32, copy=True)
    masked[:, tgt] = NEG_LARGE
    m = masked.max(axis=1, keepdims=True)
    e = np.exp(masked - m, dtype=np.float32)
    return (e / e.sum(axis=1, keepdims=True, dtype=np.float32)).astype(np.float32)


def _build_program(reps: int = 1):
    """Build + compile the SPMD program.

    reps > 1 emits the whole body N times, serialized end-to-start via an
    explicit dependency and chained through x — used only for benchmarking
    per-execution device time with dispatch overhead amortized out.
    """
    import concourse.bacc as bacc
    import concourse.bass as bass
    import concourse.mybir as mybir
    import concourse.tile as tile

    f32 = mybir.dt.float32
    nc = bacc.Bacc(
        "TRN2",
        target_bir_lowering=False,
        debug=False,
        enable_asserts=False,
        num_devices=NC,
    )
    t_shard = nc.dram_tensor("t_shard", [SH, N, N], f32, kind="ExternalInput").ap()
    q_cols = nc.dram_tensor("q_cols", [N, SH], f32, kind="ExternalInput").ap()
    x0c = nc.dram_tensor("x0c", [128, 4], f32, kind="ExternalInput").ap()
    x_out = nc.dram_tensor("x_out", [128, 4], f32, kind="ExternalOutput").ap()

    with tile.TileContext(nc) as tc:
        with (
            tc.tile_pool(name="const", bufs=1) as cpool,
            tc.tile_pool(name="tbuf", bufs=3) as tpool,
            tc.tile_pool(name="sfull", bufs=1) as spool,
            tc.tile_pool(name="xbuf", bufs=2) as xpool,
            tc.tile_pool(name="psum_s", bufs=4, space="PSUM") as pspool,
            tc.tile_pool(name="psum_w", bufs=2, space="PSUM") as pwpool,
            tc.tile_pool(name="dram", bufs=1, space="DRAM") as dpool,
        ):
            # Qi columns for this core's a-shard: q_cols[b, a] = Qi[a0+a, b]
            qts = []
            for g in range(4):
                qt = cpool.tile([128, SH], f32, tag=f"qt{g}")
                nc.sync.dma_start(qt[:], q_cols[g * 128:(g + 1) * 128, :])
                qts.append(qt)

            xc = None
            prev_tail = None  # last instruction of previous rep (bench mode)
            for rep in range(reps):
                ag_in = dpool.tile([SH, N], f32, tag=f"ag_in{rep}")
                ag_out = dpool.tile([N, N], f32, tag=f"ag_out{rep}")

                # ---- one pass over the T shard:
                # S[a,:] = sum_b Qi[a,b] * T[a,b,:].  S rows come out of the
                # PE as [1, 512] on partition 0; compute engines can only
                # write 32-aligned partition offsets, so stage SROWS of them
                # side-by-side in the free dim and DMA to DRAM.
                SROWS = 8
                for ib in range(SH // AB):
                    tt = tpool.tile([128, AB * 4 * N], f32, tag="tt")
                    src = t_shard[ib * AB:(ib + 1) * AB]  # [AB, N, N]
                    ld = nc.sync.dma_start(
                        tt[:].rearrange("p (j g k) -> p j g k", j=AB, g=4),
                        src.rearrange("j (g p) k -> p j g k", g=4, p=128),
                    )
                    if ib == 0 and prev_tail is not None:
                        bass._add_dep_helper(
                            ld.ins, prev_tail.ins, True, "serialize bench rep")
                    for j in range(AB):
                        a = ib * AB + j
                        if a % SROWS == 0:
                            stage = cpool.tile([1, SROWS * N], f32,
                                               tag="stage", bufs=3)
                        ps = pspool.tile([1, N], f32, tag="ps")
                        for g in range(4):
                            nc.tensor.matmul(
                                ps[:],
                                lhsT=qts[g][:, a:a + 1],
                                rhs=tt[:, (j * 4 + g) * N:(j * 4 + g + 1) * N],
                                start=(g == 0),
                                stop=(g == 3),
                            )
                        r = a % SROWS
                        nc.vector.tensor_copy(stage[:, r * N:(r + 1) * N],
                                              ps[:])
                        if r == SROWS - 1:
                            nc.sync.dma_start(
                                ag_in[a - r:a + 1, :].rearrange(
                                    "(p r) k -> p r k", p=1),
                                stage[:].rearrange("p (r k) -> p r k",
                                                   r=SROWS),
                            )

                # ---- AllGather the S shards into the full [512, 512] S
                nc.gpsimd.collective_compute(
                    "AllGather",
                    mybir.AluOpType.bypass,
                    replica_groups=[list(range(NC))],
                    ins=[ag_in.opt()],
                    outs=[ag_out.opt()],
                )
                sf = []
                for g in range(4):
                    t = spool.tile([128, N], f32, tag=f"sf{g}")
                    nc.sync.dma_start(t[:], ag_out[g * 128:(g + 1) * 128, :])
                    sf.append(t)

                # ---- 8 waves: x = relu(x + x @ S), x column-major [128, 4]
                if xc is None:
                    xc = xpool.tile([128, 4], f32, tag="xc")
                    nc.sync.dma_start(xc[:], x0c[:])
                tail = None
                for _w in range(NWAVES):
                    po = pwpool.tile([128, 4], f32, tag="po")
                    for g in range(4):        # output k-chunk
                        for ac in range(4):   # contraction a-chunk
                            nc.tensor.matmul(
                                po[:, g:g + 1],
                                lhsT=sf[ac][:, g * 128:(g + 1) * 128],
                                rhs=xc[:, ac:ac + 1],
                                start=(ac == 0),
                                stop=(ac == 3),
                            )
                    xn = xpool.tile([128, 4], f32, tag="xc")
                    nc.vector.tensor_add(xn[:], xc[:], po[:])
                    tail = nc.vector.tensor_relu(xn[:], xn[:])
                    xc = xn
                prev_tail = tail
            nc.sync.dma_start(x_out[:], xc[:])

    nc.compile()
    return nc


def _in_maps(x_init: np.ndarray, Qi: np.ndarray, T: np.ndarray):
    x0c = np.ascontiguousarray(
        x_init.astype(np.float32).reshape(4, 128).T)  # x0c[p, g] = x[g*128+p]
    return [
        {
            "t_shard": T[c * SH:(c + 1) * SH],
            "q_cols": np.ascontiguousarray(Qi[c * SH:(c + 1) * SH].T),
            "x0c": x0c,
        }
        for c in range(NC)
    ]


def get_program(reps: int = 1):
    if reps not in _prog_cache:
        _prog_cache[reps] = _build_program(reps)
    return _prog_cache[reps]


def _run_device(x_init: np.ndarray, Qi: np.ndarray, T: np.ndarray) -> np.ndarray:
    # No NTFF hook exists in this chipless client; a stray BASS_TRACE=1
    # in the environment would crash run_bass_kernel_spmd otherwise.
    import os
    os.environ.setdefault("BASS_NEVER_TRACE", "1")
    import concourse.bass_utils as bass_utils
    global last_results

    nc = get_program()
    res = bass_utils.run_bass_kernel_spmd(
        nc, _in_maps(x_init, Qi, T), core_ids=list(range(NC)))
    last_results = res
    out = res.results[0]["x_out"]  # [128, 4]
    return np.ascontiguousarray(out.T).reshape(N).astype(np.float32)


def _reference_numpy(x0, logits, T, tgt):
    """Faithful per-wave fallback (any input values), pure numpy."""
    x = np.maximum(np.asarray(x0, dtype=np.float32), 0.0)
    logits = np.asarray(logits, dtype=np.float32)
    Tf = np.asarray(T, dtype=np.float32).reshape(N * N, N)
    for _ in range(NWAVES):
        p1 = x > 0.0
        p2 = p1.copy()
        p2[tgt] = False
        allowed = p1[:, None] & p2[None, :]
        masked = np.where(allowed, logits, np.float32(NEG_LARGE))
        m = masked.max(axis=1, keepdims=True)
        e = np.exp(masked - m, dtype=np.float32)
        probs = e / e.sum(axis=1, keepdims=True, dtype=np.float32)
        cnt = allowed.sum(axis=1, keepdims=True)
        Qi = np.where(cnt > 0, probs, np.float32(0.0)).astype(np.float32)
        w = (x[:, None] * Qi).reshape(N * N)
        offspring = w @ Tf
        x = np.maximum(x + offspring, 0.0).astype(np.float32)
    return x


def kernel(x0, logits, T, target_idx) -> np.ndarray:
    x0 = np.asarray(x0)
    logits = np.asarray(logits, dtype=np.float32)
    T = np.ascontiguousarray(np.asarray(T, dtype=np.float32))
    tgt = int(np.asarray(target_idx).ravel()[0])

    x_init = np.maximum(x0.astype(np.float32), 0.0)
    # Fast path requires the presence mask to be wave-invariant: guaranteed
    # when every x0 > 0 and T >= 0 (offspring >= 0 keeps x > 0 forever).
    if bool(np.all(x_init > 0.0)) and float(T.min()) >= 0.0:
        Qi = _qi_matrix(logits, tgt)
        return _run_device(x_init, Qi, T)
    return _reference_numpy(x0, logits, T, tgt)
